# revision 1
# baseline (speedup 1.0000x reference)
"""MLA prefill kernel for TRN2, 8 NeuronCores.

Sharding: data-parallel over 128-row query blocks. Flattened rows are
[B*S] = 4096 = 2 batches x 16 blocks of 128. Core c (batch b=c//4, j=c%4)
owns blocks {j, 7-j, 8+j, 15-j} of its batch, so every core runs an
identical program: q-slots 0,1 attend keys [0,1024), slots 2,3 attend
[0,2048), with host-built additive masks supplying exact causality.

Per core: x_c -> (down-proj -> LN -> up-proj + RoPE) for Q and KV of its
own rows; K^T/V are AllGathered within each batch group of 4 cores; then
causal attention and the output projection run fully locally.

All matmuls in bf16 (fp32 PSUM accumulation); LN/softmax statistics fp32.
"""

import math

import numpy as np
import ml_dtypes

import concourse.bass as bass
import concourse.tile as tile
import concourse.mybir as mybir
from concourse import bacc
from concourse.bass_utils import run_bass_kernel_spmd

BF16 = mybir.dt.bfloat16
F32 = mybir.dt.float32
NP_BF16 = ml_dtypes.bfloat16

B, S, D = 2, 2048, 2048
H, DH = 16, 128
P = 128
NCORES = 8
RPC = 512          # rows per core
ROPE_THETA = 10000.0
LN_EPS = 1e-5
NEG = -30000.0
NK = (1024, 1024, 2048, 2048)     # key extent per q-slot
MOFF = (0, 1024, 2048, 4096)      # mask column offset per q-slot
MASK_COLS = 6144
KT_COLS = H * RPC                  # 8192: col = h*512 + slot*128 + kk
V_OFF = KT_COLS                    # V region: col = V_OFF + slot*2048 + h*128 + dd
KV_COLS = 2 * KT_COLS              # 16384

AF = mybir.ActivationFunctionType
ALU = mybir.AluOpType


def _blocks(c):
    j = c % 4
    return [j, 7 - j, 8 + j, 15 - j]


def _rank_slot(i):
    """Batch-local key block i (0..15) -> (rank offset in batch group, slot)."""
    if i < 4:
        return i, 0
    if i < 8:
        return 7 - i, 1
    if i < 12:
        return i - 8, 2
    return 15 - i, 3


# ---------------------------------------------------------------- emission


def _emit(nc, tc, t_in, t_out):
    x = t_in["x"].ap()
    wdq = t_in["wdq"].ap()
    wuq = t_in["wuq"].ap()
    wdkv = t_in["wdkv"].ap()
    wukv = t_in["wukv"].ap()
    wot = t_in["wot"].ap()
    gq = t_in["gq"].ap()
    bq = t_in["bq"].ap()
    gkv = t_in["gkv"].ap()
    bkv = t_in["bkv"].ap()
    cosq = t_in["cosq"].ap()
    sinq = t_in["sinq"].ap()
    cosk = t_in["cosk"].ap()
    sink = t_in["sink"].ap()
    masks = t_in["masks"].ap()
    ident_d = t_in["ident"].ap()
    out_d = t_out["out"].ap()
    ckv_d = t_out["ckv"].ap()

    with (
        tc.tile_pool(name="consts", bufs=1) as consts,
        tc.tile_pool(name="big", bufs=1) as big,
        tc.tile_pool(name="actT", bufs=1) as actT,
        tc.tile_pool(name="stat", bufs=8) as stat,
        tc.tile_pool(name="dram", bufs=1, space="DRAM") as dram,
    ):
        ident = consts.tile([P, P], BF16)
        nc.sync.dma_start(ident[:], ident_d[:])

        xT = big.tile([P, 16, RPC], BF16, tag="xT")
        qT = big.tile([P, H, RPC], BF16, tag="qT")
        oT = big.tile([P, H, RPC], BF16, tag="oT")

        kv_in = dram.tile([P, KV_COLS], BF16)
        kv_out = dram.tile([4 * P, KV_COLS], BF16)

        # ---- transpose x into xT [d, rows] via DMA xbar (bf16) -----------
        # dst [pi=128, po=16, rows=512] <- src x viewed [rows, po, pi]
        nc.sync.dma_start_transpose(
            xT[:], x.rearrange("n (po pi) -> n po pi", pi=P)
        )

        def down_ln(w_dram, gamma_d, beta_d, lnout_bf, ckv_dma):
            """t = x @ W ; LN -> (optional ckv DMA out) + bf16 copy."""
            with (
                tc.tile_pool(name="wbig", bufs=2) as wbig,
                tc.tile_pool(name="mm", bufs=4, space="PSUM") as mm,
                tc.tile_pool(name="lnstage", bufs=1) as lnstage,
                tc.tile_pool(name="lnscratch", bufs=2) as lnscratch,
                tc.tile_pool(name="gb", bufs=1) as gb,
            ):
                t_sb = lnstage.tile([P, 4, D], F32)
                g_bc = gb.tile([P, D], BF16, tag="g")
                b_bc = gb.tile([P, D], BF16, tag="b")
                nc.sync.dma_start(g_bc[:], gamma_d[:])
                nc.sync.dma_start(b_bc[:], beta_d[:])
                for cc in range(4):
                    w_cc = wbig.tile([P, 16, 512], BF16, tag="w")
                    for kt in range(16):
                        nc.sync.dma_start(
                            w_cc[:, kt, :],
                            w_dram[kt * P : (kt + 1) * P, cc * 512 : (cc + 1) * 512],
                        )
                    for rt in range(4):
                        ps = mm.tile([P, 512], F32)
                        for kt in range(16):
                            nc.tensor.matmul(
                                ps,
                                xT[:, kt, rt * P : (rt + 1) * P],
                                w_cc[:, kt, :],
                                start=(kt == 0),
                                stop=(kt == 15),
                            )
                        nc.vector.tensor_copy(
                            t_sb[:, rt, cc * 512 : (cc + 1) * 512], ps
                        )
                for rt in range(4):
                    row = t_sb[:, rt, :]
                    ssum = stat.tile([P, 1], F32, tag="s")
                    nmu = stat.tile([P, 1], F32, tag="s")
                    nc.vector.reduce_sum(ssum, row, axis=mybir.AxisListType.X)
                    nc.vector.tensor_scalar_mul(nmu, ssum, -1.0 / D)
                    nc.vector.tensor_scalar_add(row, row, nmu)
                    sq = lnscratch.tile([P, D], F32, tag="sq")
                    ssq = stat.tile([P, 1], F32, tag="s")
                    nc.scalar.activation(sq, row, AF.Square, accum_out=ssq)
                    veps = stat.tile([P, 1], F32, tag="s")
                    nc.vector.tensor_scalar(
                        veps, ssq, 1.0 / D, LN_EPS, ALU.mult, ALU.add
                    )
                    std = stat.tile([P, 1], F32, tag="s")
                    nc.scalar.activation(std, veps, AF.Sqrt)
                    rstd = stat.tile([P, 1], F32, tag="s")
                    nc.vector.reciprocal(rstd, std)
                    nc.vector.tensor_scalar_mul(row, row, rstd)
                    nc.vector.tensor_tensor(row, row, g_bc, ALU.mult)
                    nc.vector.tensor_tensor(row, row, b_bc, ALU.add)
                    if ckv_dma:
                        nc.sync.dma_start(ckv_d[rt * P : (rt + 1) * P, :], row)
                    nc.vector.tensor_copy(lnout_bf[:, rt, :], row)

        def transpose_act(src_bf, dst_T):
            with tc.tile_pool(name="trp2", bufs=4, space="PSUM") as trp2:
                for rt in range(4):
                    for kt in range(16):
                        ps = trp2.tile([P, P], BF16)
                        nc.tensor.transpose(
                            ps, src_bf[:, rt, kt * P : (kt + 1) * P], ident
                        )
                        nc.vector.tensor_copy(dst_T[:, kt, rt * P : (rt + 1) * P], ps)

        def rope_head(ps, cos_t, sin_t, dst_bf, rope_pool):
            """dst = ps*cos + rotate_half(ps)*sin_signed  (free dim = 512 rows)."""
            q_raw = rope_pool.tile([P, RPC], F32, tag="qr")
            nc.vector.tensor_copy(q_raw, ps)
            rot = rope_pool.tile([P, RPC], F32, tag="rot")
            nc.sync.dma_start(rot[0:64, :], q_raw[64:128, :])
            nc.sync.dma_start(rot[64:128, :], q_raw[0:64, :])
            acc = rope_pool.tile([P, RPC], F32, tag="acc")
            nc.vector.tensor_tensor(acc, q_raw, cos_t, ALU.mult)
            t2 = rope_pool.tile([P, RPC], F32, tag="t2")
            nc.vector.tensor_tensor(t2, rot, sin_t, ALU.mult)
            nc.vector.tensor_tensor(dst_bf, t2, acc, ALU.add)

        # ================= KV path =================
        ckvb = actT.tile([P, 4, D], BF16, tag="lnb")
        down_ln(wdkv, gkv, bkv, ckvb, ckv_dma=True)
        ckvT = actT.tile([P, 16, RPC], BF16, tag="aT")
        transpose_act(ckvb, ckvT)

        # K^T per head + RoPE -> kv_in
        with (
            tc.tile_pool(name="whead", bufs=2) as whead,
            tc.tile_pool(name="mm2", bufs=2, space="PSUM") as mm2,
            tc.tile_pool(name="rope", bufs=2) as rope_pool,
            tc.tile_pool(name="cs", bufs=1) as cs,
            tc.tile_pool(name="evict", bufs=3) as evict,
        ):
            cosk_t = cs.tile([P, RPC], F32, tag="ck")
            sink_t = cs.tile([P, RPC], F32, tag="sk")
            nc.sync.dma_start(cosk_t[:], cosk[:])
            nc.sync.dma_start(sink_t[:], sink[:])
            for h in range(H):
                w_h = whead.tile([P, 16, P], BF16, tag="wh")
                for kt in range(16):
                    nc.sync.dma_start(
                        w_h[:, kt, :],
                        wukv[kt * P : (kt + 1) * P, h * P : (h + 1) * P],
                    )
                ps = mm2.tile([P, RPC], F32)
                for kt in range(16):
                    nc.tensor.matmul(
                        ps, w_h[:, kt, :], ckvT[:, kt, :],
                        start=(kt == 0), stop=(kt == 15),
                    )
                k_bf = evict.tile([P, RPC], BF16, tag="kb")
                rope_head(ps, cosk_t, sink_t, k_bf, rope_pool)
                nc.sync.dma_start(kv_in[:, h * RPC : (h + 1) * RPC], k_bf)

            # V (natural layout) -> kv_in
            with tc.tile_pool(name="wbig2", bufs=2) as wbig2:
                for cc in range(4):
                    w_cc = wbig2.tile([P, 16, 512], BF16, tag="wv")
                    for kt in range(16):
                        nc.sync.dma_start(
                            w_cc[:, kt, :],
                            wukv[kt * P : (kt + 1) * P,
                                 D + cc * 512 : D + (cc + 1) * 512],
                        )
                    for slot in range(4):
                        psv = mm2.tile([P, 512], F32, tag="psv")
                        for kt in range(16):
                            nc.tensor.matmul(
                                psv,
                                ckvT[:, kt, slot * P : (slot + 1) * P],
                                w_cc[:, kt, :],
                                start=(kt == 0), stop=(kt == 15),
                            )
                        v_bf = evict.tile([P, 512], BF16, tag="vb")
                        nc.vector.tensor_copy(v_bf, psv)
                        nc.sync.dma_start(
                            kv_in[:, V_OFF + slot * 2048 + cc * 512 :
                                  V_OFF + slot * 2048 + (cc + 1) * 512],
                            v_bf,
                        )

        # AllGather K^T/V within each batch group of 4 cores.
        import os as _os
        if _os.environ.get("BASS_MLA_NO_CC"):
            # Timing-sim variant: same byte volume into kv_out, no collective.
            for r in range(4):
                nc.sync.dma_start(kv_out[r * P : (r + 1) * P, :], kv_in[:])
        else:
            nc.gpsimd.collective_compute(
                "AllGather",
                ALU.bypass,
                replica_groups=[[0, 1, 2, 3], [4, 5, 6, 7]],
                ins=[kv_in.opt()],
                outs=[kv_out.opt()],
            )

        # ================= Q path (overlaps the gather) =================
        cqb = actT.tile([P, 4, D], BF16, tag="lnb")
        down_ln(wdq, gq, bq, cqb, ckv_dma=False)
        cqT = actT.tile([P, 16, RPC], BF16, tag="aT")
        transpose_act(cqb, cqT)

        with (
            tc.tile_pool(name="whead2", bufs=2) as whead2,
            tc.tile_pool(name="mm3", bufs=2, space="PSUM") as mm3,
            tc.tile_pool(name="rope2", bufs=2) as rope2,
            tc.tile_pool(name="cs2", bufs=1) as cs2,
        ):
            cosq_t = cs2.tile([P, RPC], F32, tag="cq")
            sinq_t = cs2.tile([P, RPC], F32, tag="sq")
            nc.sync.dma_start(cosq_t[:], cosq[:])
            nc.sync.dma_start(sinq_t[:], sinq[:])
            for h in range(H):
                w_h = whead2.tile([P, 16, P], BF16, tag="wh")
                for kt in range(16):
                    nc.sync.dma_start(
                        w_h[:, kt, :],
                        wuq[kt * P : (kt + 1) * P, h * P : (h + 1) * P],
                    )
                ps = mm3.tile([P, RPC], F32)
                for kt in range(16):
                    nc.tensor.matmul(
                        ps, w_h[:, kt, :], cqT[:, kt, :],
                        start=(kt == 0), stop=(kt == 15),
                    )
                rope_head(ps, cosq_t, sinq_t, qT[:, h, :], rope2)

        # ================= attention =================
        with (
            tc.tile_pool(name="att", bufs=3) as att,
            tc.tile_pool(name="ptp", bufs=6) as ptp,
            tc.tile_pool(name="mcol", bufs=1) as mcol,
            tc.tile_pool(name="scp", bufs=2, space="PSUM") as scp,
            tc.tile_pool(name="trp3", bufs=4, space="PSUM") as trp3,
            tc.tile_pool(name="avp", bufs=2, space="PSUM") as avp,
        ):
            mask_sb = mcol.tile([P, MASK_COLS], BF16)
            nc.sync.dma_start(mask_sb[:], masks[:])
            for h in range(H):
                kt_bh = att.tile([P, 16, P], BF16, tag="kt")
                v_bh = att.tile([P, 16, P], BF16, tag="v")
                for i in range(16):
                    ro, sl = _rank_slot(i)
                    nc.sync.dma_start(
                        kt_bh[:, i, :],
                        kv_out[ro * P : (ro + 1) * P,
                               h * RPC + sl * P : h * RPC + (sl + 1) * P],
                    )
                    nc.sync.dma_start(
                        v_bh[:, i, :],
                        kv_out[ro * P : (ro + 1) * P,
                               V_OFF + sl * 2048 + h * P : V_OFF + sl * 2048 + (h + 1) * P],
                    )
                for qs in range(4):
                    nk = NK[qs]
                    nkb = nk // P
                    sc = att.tile([P, 2048], F32, tag="sc")
                    for ch in range(nk // 512):
                        ps = scp.tile([P, 512], F32)
                        nc.tensor.matmul(
                            ps,
                            qT[:, h, qs * P : (qs + 1) * P],
                            kt_bh[:, ch * 4 : (ch + 1) * 4, :],
                            start=True, stop=True,
                        )
                        nc.vector.scalar_tensor_tensor(
                            sc[:, ch * 512 : (ch + 1) * 512],
                            ps,
                            60.0,
                            mask_sb[:, MOFF[qs] + ch * 512 : MOFF[qs] + (ch + 1) * 512],
                            ALU.min,
                            ALU.add,
                        )
                    p_f = att.tile([P, 2048], F32, tag="pf")
                    lsum = stat.tile([P, 1], F32, tag="l")
                    nc.scalar.activation(
                        p_f[:, :nk], sc[:, :nk], AF.Exp, accum_out=lsum
                    )
                    rl = stat.tile([P, 1], F32, tag="l")
                    nc.vector.reciprocal(rl, lsum)
                    pb = att.tile([P, 2048], BF16, tag="pb")
                    # normalize+cast on ScalarE (per-partition scale AP) to
                    # keep DVE free for psum evictions and pt copies
                    nc.scalar.activation(pb[:, :nk], p_f[:, :nk], AF.Copy, scale=rl)
                    ops = avp.tile([P, P], F32)
                    for kb in range(nkb):
                        tp = trp3.tile([P, P], BF16)
                        nc.tensor.transpose(tp, pb[:, kb * P : (kb + 1) * P], ident)
                        pt = ptp.tile([P, P], BF16, tag="pt")
                        nc.vector.tensor_copy(pt, tp)
                        nc.tensor.matmul(
                            ops, v_bh[:, kb, :], pt,
                            start=(kb == 0), stop=(kb == nkb - 1),
                        )
                    nc.vector.tensor_copy(oT[:, h, qs * P : (qs + 1) * P], ops)

        # ================= output projection =================
        with (
            tc.tile_pool(name="wbig3", bufs=2) as wbig3,
            tc.tile_pool(name="mm4", bufs=4, space="PSUM") as mm4,
            tc.tile_pool(name="oev", bufs=3) as oev,
        ):
            for cc in range(4):
                w_cc = wbig3.tile([P, 16, 512], BF16, tag="wo")
                for kt in range(16):
                    nc.sync.dma_start(
                        w_cc[:, kt, :],
                        wot[kt * P : (kt + 1) * P, cc * 512 : (cc + 1) * 512],
                    )
                for rt in range(4):
                    ps = mm4.tile([P, 512], F32)
                    for kt in range(16):
                        nc.tensor.matmul(
                            ps,
                            oT[:, kt, rt * P : (rt + 1) * P],
                            w_cc[:, kt, :],
                            start=(kt == 0), stop=(kt == 15),
                        )
                    o_sb = oev.tile([P, 512], F32, tag="ob")
                    nc.vector.tensor_copy(o_sb, ps)
                    nc.sync.dma_start(
                        out_d[rt * P : (rt + 1) * P, cc * 512 : (cc + 1) * 512],
                        o_sb,
                    )


# ---------------------------------------------------------------- build


_CACHE = {}


def _build():
    if "nc" in _CACHE:
        return _CACHE["nc"]
    nc = bacc.Bacc("TRN2", target_bir_lowering=False, debug=False, num_devices=NCORES)
    t_in = {}

    def inp(name, shape, dt):
        t_in[name] = nc.dram_tensor(name, shape, dt, kind="ExternalInput")

    inp("x", [RPC, D], BF16)
    inp("wdq", [D, D], BF16)
    inp("wuq", [D, D], BF16)
    inp("wdkv", [D, D], BF16)
    inp("wukv", [D, 2 * D], BF16)
    inp("wot", [D, D], BF16)
    inp("gq", [P, D], BF16)
    inp("bq", [P, D], BF16)
    inp("gkv", [P, D], BF16)
    inp("bkv", [P, D], BF16)
    inp("cosq", [P, RPC], F32)
    inp("sinq", [P, RPC], F32)
    inp("cosk", [P, RPC], F32)
    inp("sink", [P, RPC], F32)
    inp("masks", [P, MASK_COLS], BF16)
    inp("ident", [P, P], BF16)
    t_out = {
        "out": nc.dram_tensor("out", [RPC, D], F32, kind="ExternalOutput"),
        "ckv": nc.dram_tensor("ckv", [RPC, D], F32, kind="ExternalOutput"),
    }
    with tile.TileContext(nc) as tc:
        _emit(nc, tc, t_in, t_out)
    nc.finalize()
    _CACHE["nc"] = nc
    return nc


# ---------------------------------------------------------------- host


def host_prep(inputs):
    """Build the 8 per-core input maps (numpy) from full inputs."""
    x = np.asarray(inputs["x"], np.float32).reshape(B * S, D)
    wdq = np.asarray(inputs["W_dq"], np.float32).astype(NP_BF16)
    wuq = np.asarray(inputs["W_uq"], np.float32).astype(NP_BF16)
    wdkv = np.asarray(inputs["W_dkv"], np.float32).astype(NP_BF16)
    wukv = np.asarray(inputs["W_ukv"], np.float32).astype(NP_BF16)
    wot = np.ascontiguousarray(np.asarray(inputs["W_o"], np.float32).T).astype(NP_BF16)

    def bc(v):
        return np.ascontiguousarray(
            np.broadcast_to(np.asarray(v, np.float32), (P, D))
        ).astype(NP_BF16)

    gq, bq = bc(inputs["q_gamma"]), bc(inputs["q_beta"])
    gkv, bkv = bc(inputs["kv_gamma"]), bc(inputs["kv_beta"])

    freqs = 1.0 / (ROPE_THETA ** (np.arange(0, DH, 2, dtype=np.float32) / DH))
    t = np.arange(S, dtype=np.float32)
    emb = np.outer(t, freqs)                      # [S, 64]
    cos = np.concatenate([np.cos(emb), np.cos(emb)], -1).T.astype(np.float32)  # [128,S]
    sin = np.concatenate([np.sin(emb), np.sin(emb)], -1).T.astype(np.float32)
    sin_signed = sin.copy()
    sin_signed[:64] *= -1.0
    scale = 1.0 / math.sqrt(DH)

    ident = np.eye(P, dtype=np.float32).astype(NP_BF16)

    in_maps = []
    for c in range(NCORES):
        b = c // 4
        blks = _blocks(c)
        rows = np.concatenate(
            [np.arange(blk * P, (blk + 1) * P) for blk in blks]
        )
        pos = rows  # positions within the batch
        x_c = np.ascontiguousarray(x[b * S + rows]).astype(NP_BF16)
        mask = np.full((P, MASK_COLS), NEG, np.float32)
        for qs, blk in enumerate(blks):
            nk = NK[qs]
            qpos = blk * P + np.arange(P)[:, None]
            kpos = np.arange(nk)[None, :]
            mask[:, MOFF[qs] : MOFF[qs] + nk] = np.where(kpos <= qpos, 0.0, NEG)
        in_maps.append(
            {
                "x": x_c,
                "wdq": wdq, "wuq": wuq, "wdkv": wdkv, "wukv": wukv, "wot": wot,
                "gq": gq, "bq": bq, "gkv": gkv, "bkv": bkv,
                "cosq": np.ascontiguousarray(cos[:, pos] * scale),
                "sinq": np.ascontiguousarray(sin_signed[:, pos] * scale),
                "cosk": np.ascontiguousarray(cos[:, pos]),
                "sink": np.ascontiguousarray(sin_signed[:, pos]),
                "masks": mask.astype(NP_BF16),
                "ident": ident,
            }
        )
    return in_maps


def host_unshard(results):
    out = np.zeros((B * S, D), np.float32)
    ckv = np.zeros((B * S, D), np.float32)
    for c in range(NCORES):
        b = c // 4
        for qs, blk in enumerate(_blocks(c)):
            g = b * S + blk * P
            out[g : g + P] = results[c]["out"][qs * P : (qs + 1) * P]
            ckv[g : g + P] = results[c]["ckv"][qs * P : (qs + 1) * P]
    return out.reshape(B, S, D), ckv.reshape(B, S, D)


def kernel(**inputs):
    nc = _build()
    in_maps = host_prep(inputs)
    res = run_bass_kernel_spmd(nc, in_maps, core_ids=list(range(NCORES)))
    return host_unshard(res.results)


if __name__ == "__main__":
    rng = np.random.default_rng(0)
    ins = {
        "x": rng.standard_normal((B, S, D), np.float32),
        "W_dq": 0.02 * rng.standard_normal((D, D), np.float32),
        "W_uq": 0.02 * rng.standard_normal((D, D), np.float32),
        "q_gamma": np.ones(D, np.float32),
        "q_beta": np.zeros(D, np.float32),
        "W_dkv": 0.02 * rng.standard_normal((D, D), np.float32),
        "W_ukv": 0.02 * rng.standard_normal((D, 2 * D), np.float32),
        "kv_gamma": np.ones(D, np.float32),
        "kv_beta": np.zeros(D, np.float32),
        "W_o": 0.02 * rng.standard_normal((D, D), np.float32),
    }
    o, ck = kernel(**ins)
    print(o.shape, ck.shape, float(np.abs(o).mean()), float(np.abs(ck).mean()))

